# revision 10
# baseline (speedup 1.0000x reference)
"""DynamicGAT (2-layer GAT + graph-LN + mean-pool head) on 8 TRN2 NeuronCores.

Strategy (dst-sharded message passing):
  - Host (numpy, integer/layout work only): add self loops, balance nodes
    into 128-slot dst tiles (greedy LPT on in-degree), permute node ids,
    bucket edges per dst tile padded to Cmax chunks of 128 edges, build
    per-chunk one-hot (edge -> dst-slot) int8 matrices, graph one-hots,
    and broadcast-materialized weight tiles.
  - Device, per layer:
      Phase A (own node slice): h = x @ W (PE, via PE-transpose of x),
        attention logit halves al_s/al_d (DVE mul+reduce), pack
        [h:bf16[512] | al_s:f32[8]] rows, write local table + al_d table.
      AllGather the packed table (8 cores, ~7MB/rank).
      Phase B (own dst tiles): batched indirect-DMA gather of packed rows
        by edge src, gather al_d rows by edge dst, per-chunk softmax
        (no max-subtraction; logits are O(1)) and exp-weighted rows,
        PSUM-accumulated one-hot matmul aggregation (values + denominator),
        then normalize, head-mean, bias, residual.
      Graph LayerNorm: per-graph sums via one-hot matmul into a persistent
        PSUM accumulator, tiny AllReduce, per-node broadcast via transposed
        one-hot matmul, normalize + affine + ReLU (DVE).
  - Head: per-graph mean-pool via one-hot matmul + AllReduce, tiny matmul,
    bias + ReLU.
"""

import math
import os
import sys

import numpy as np

sys.path.insert(0, "/opt/trn_rl_repo")

P = 128


def full_cfg():
    return dict(
        N=50000, E=400000, G=64, DIM=64, HEADS=8, OUT=32,
        NC=8, TPC=49, NEG=0.2, EPS=1e-5,
    )


# ---------------------------------------------------------------- host prep

def _balance_nodes(deg, T):
    """Assign nodes to T tiles of <=P slots, balancing total in-degree.
    Returns slot_of_node [N] (global slot id = tile*P + slot)."""
    import heapq
    N = deg.shape[0]
    order = np.argsort(-deg, kind="stable")
    heap = [(0, t) for t in range(T)]
    heapq.heapify(heap)
    counts = np.zeros(T, np.int64)
    slot_of_node = np.empty(N, np.int64)
    for n in order:
        while True:
            load, t = heapq.heappop(heap)
            if counts[t] < P:
                break
        slot_of_node[n] = t * P + counts[t]
        counts[t] += 1
        if counts[t] < P:
            heapq.heappush(heap, (load + int(deg[n]), t))
    return slot_of_node


def prep(cfg, x, edge_index, batch, weights):
    """weights: dict with W1,a_src1,a_dst1,b1,ln1_w,ln1_b, W2,..., Wh,bh.
    Returns (in_maps, Cmax)."""
    N, E, G = cfg["N"], cfg["E"], cfg["G"]
    NC, TPC, DIM, H = cfg["NC"], cfg["TPC"], cfg["DIM"], cfg["HEADS"]
    HD = H * DIM
    T = NC * TPC
    NSLOT = T * P
    SPC = TPC * P  # slots per core

    x = np.asarray(x, np.float32)
    ei = np.asarray(edge_index, np.int64)
    batch = np.asarray(batch, np.int64)

    loops = np.arange(N, dtype=np.int64)
    src = np.concatenate([ei[0], loops])
    dst = np.concatenate([ei[1], loops])
    deg = np.bincount(dst, minlength=N)

    slot_of_node = _balance_nodes(deg, T)

    # per-tile edge buckets (Cmax chunks of 128 edges, zero-padded with src slot 0)
    e_tile = slot_of_node[dst] // P
    ord_e = np.argsort(e_tile, kind="stable")
    src_s, dst_s = src[ord_e], dst[ord_e]
    e_tile_s = e_tile[ord_e]
    tile_cnt = np.bincount(e_tile_s, minlength=T)
    CT = int(math.ceil(tile_cnt.max() / P))

    gidx = np.zeros((T, P, CT), np.int32)     # src slot per (edge p, chunk c)
    oh = np.zeros((T, P, CT * P), np.int8)    # [edge p, c*128 + dloc]
    ohT = np.zeros((T, P, CT * P), np.int8)   # [dloc, c*128 + edge p]
    starts = np.concatenate([[0], np.cumsum(tile_cnt)])
    for t in range(T):
        s, e = starts[t], starts[t + 1]
        sslot = slot_of_node[src_s[s:e]]
        dloc = slot_of_node[dst_s[s:e]] % P
        k = np.arange(e - s)
        c, p = k // P, k % P
        gidx[t, p, c] = sslot
        oh[t, p, c * P + dloc] = 1
        ohT[t, dloc, c * P + p] = 1

    # graph one-hots (zero rows for empty slots)
    node_of_slot = np.full(NSLOT, -1, np.int64)
    node_of_slot[slot_of_node] = np.arange(N)
    goh = np.zeros((T, P, G), np.float32)
    tt, ss = np.divmod(np.arange(NSLOT), P)
    valid = node_of_slot >= 0
    goh[tt[valid], ss[valid], batch[node_of_slot[valid]]] = 1.0
    gohT = np.ascontiguousarray(np.transpose(goh, (0, 2, 1)))

    xp = np.zeros((NSLOT, DIM), np.float32)
    xp[slot_of_node] = x

    cnt = np.bincount(batch, minlength=G).astype(np.float32).reshape(G, 1)

    def bc_row(a, reps=P):  # [k] -> [reps, k]
        return np.ascontiguousarray(
            np.broadcast_to(np.asarray(a, np.float32).reshape(1, -1), (reps, a.size)))

    w = {k: np.asarray(v, np.float32) for k, v in weights.items()}
    common = dict(
        W1t=w["W1"], W2t=w["W2"],
        as1=bc_row(w["a_src1"].reshape(HD)), ad1=bc_row(w["a_dst1"].reshape(HD)),
        as2=bc_row(w["a_src2"].reshape(HD)), ad2=bc_row(w["a_dst2"].reshape(HD)),
        b1t=bc_row(w["b1"]), b2t=bc_row(w["b2"]),
        lnw1=bc_row(w["ln1_w"]), lnb1=bc_row(w["ln1_b"]),
        lnw2=bc_row(w["ln2_w"]), lnb2=bc_row(w["ln2_b"]),
        Wht=w["Wh"], bht=bc_row(w["bh"], reps=G), cntt=cnt,
    )

    in_maps = []
    for c in range(NC):
        tl = slice(c * TPC, (c + 1) * TPC)
        m = dict(common)
        m["x0"] = np.ascontiguousarray(xp[c * SPC:(c + 1) * SPC])
        m["gidx"] = np.ascontiguousarray(gidx[tl])
        m["oh"] = np.ascontiguousarray(oh[tl])
        m["ohT"] = np.ascontiguousarray(ohT[tl])
        m["goh"] = np.ascontiguousarray(goh[tl])
        m["gohT"] = np.ascontiguousarray(gohT[tl])
        in_maps.append(m)
    return in_maps, CT


# ---------------------------------------------------------------- device

def build(cfg, CT):
    import concourse.bacc as bacc
    import concourse.bass as bass
    import concourse.tile as tile
    from concourse import mybir
    from concourse.masks import make_identity

    N, G = cfg["N"], cfg["G"]
    NC, TPC, DIM, H, OUT = cfg["NC"], cfg["TPC"], cfg["DIM"], cfg["HEADS"], cfg["OUT"]
    HD = H * DIM            # 512
    ALS = HD + 2 * H        # 528
    ROW = ALS               # bf16 slots per packed table row (1056B)
    T = NC * TPC
    NSLOT = T * P
    SPC = TPC * P
    f32 = mybir.dt.float32
    bf16 = mybir.dt.bfloat16
    i32 = mybir.dt.int32
    i16 = mybir.dt.int16
    i8 = mybir.dt.int8
    AX = mybir.AxisListType.X
    OP = mybir.AluOpType
    ACT = mybir.ActivationFunctionType

    nc = bacc.Bacc("TRN2", target_bir_lowering=False, debug=False,
                   enable_asserts=False, num_devices=NC)

    def din(name, shape, dt):
        return nc.dram_tensor(name, shape, dt, kind="ExternalInput")

    x0 = din("x0", [SPC, DIM], f32)
    W1t, W2t = din("W1t", [DIM, HD], f32), din("W2t", [DIM, HD], f32)
    as1, ad1 = din("as1", [P, HD], f32), din("ad1", [P, HD], f32)
    as2, ad2 = din("as2", [P, HD], f32), din("ad2", [P, HD], f32)
    b1t, b2t = din("b1t", [P, DIM], f32), din("b2t", [P, DIM], f32)
    lnw1, lnb1 = din("lnw1", [P, DIM], f32), din("lnb1", [P, DIM], f32)
    lnw2, lnb2 = din("lnw2", [P, DIM], f32), din("lnb2", [P, DIM], f32)
    Wht, bht = din("Wht", [DIM, OUT], f32), din("bht", [G, OUT], f32)
    cntt = din("cntt", [G, 1], f32)
    gxd = din("gidx", [TPC, P, CT], i32)
    ohd = din("oh", [TPC, P, CT * P], i8)
    ohTd = din("ohT", [TPC, P, CT * P], i8)
    goh = din("goh", [TPC, P, G], f32)
    gohT = din("gohT", [TPC, G, P], f32)
    outd = nc.dram_tensor("out", [G, OUT], f32, kind="ExternalOutput")

    with tile.TileContext(nc) as tc:
        with (
            tc.tile_pool(name="const", bufs=1) as cpool,
            tc.tile_pool(name="sb", bufs=3) as sb,
            tc.tile_pool(name="big", bufs=2) as big,
            tc.tile_pool(name="keep", bufs=2 * TPC) as keep,
            tc.tile_pool(name="keep2", bufs=2 * TPC) as keep2,
            tc.tile_pool(name="ps", bufs=1, space="PSUM") as ps,
            tc.tile_pool(name="acc", bufs=2, space="PSUM") as accp,
            tc.tile_pool(name="accs", bufs=2, space="PSUM") as accsp,
            tc.tile_pool(name="stat", bufs=1, space="PSUM") as statp,
            tc.tile_pool(name="dram", bufs=1, space="DRAM") as dram,
        ):
            ident = cpool.tile([P, P], f32, tag="ident")
            make_identity(nc, ident[:])

            # constants to SBUF (bf16 casts for attention vectors)
            def load_const(dt_dram, shape, dt=f32, tag=None):
                t = cpool.tile(shape, dt, tag=tag)
                nc.sync.dma_start(out=t[:], in_=dt_dram[:])
                return t

            W_sb = [load_const(W1t, [DIM, HD], tag="W1"),
                    load_const(W2t, [DIM, HD], tag="W2")]
            asad_sb = []
            for L, (a_s, a_d) in enumerate([(as1, ad1), (as2, ad2)]):
                pair = []
                for nm, dd in (("s", a_s), ("d", a_d)):
                    t32 = sb.tile([P, HD], f32, tag="acast")
                    nc.sync.dma_start(out=t32[:], in_=dd[:])
                    tb = cpool.tile([P, HD], bf16, tag=f"a{nm}{L}")
                    nc.vector.tensor_copy(out=tb[:], in_=t32[:])
                    pair.append(tb)
                asad_sb.append(pair)
            bias_sb = [load_const(b1t, [P, DIM], tag="b1"),
                       load_const(b2t, [P, DIM], tag="b2")]
            lnw_sb = [load_const(lnw1, [P, DIM], tag="lnw1"),
                      load_const(lnw2, [P, DIM], tag="lnw2")]
            lnb_sb = [load_const(lnb1, [P, DIM], tag="lnb1"),
                      load_const(lnb2, [P, DIM], tag="lnb2")]
            Wh_sb = load_const(Wht, [DIM, OUT], tag="Wh")
            bh_sb = load_const(bht, [G, OUT], tag="bh")
            cnt_sb = load_const(cntt, [G, 1], tag="cnt")
            # clipped count and 1/(cnt*DIM)
            cntc = cpool.tile([G, 1], f32, tag="cntc")
            nc.vector.tensor_scalar_max(out=cntc[:], in0=cnt_sb[:], scalar1=1.0)
            ninv = cpool.tile([G, 1], f32, tag="ninv")
            nrm = sb.tile([G, 1], f32, tag="nrm")
            nc.vector.tensor_scalar_mul(out=nrm[:], in0=cntc[:], scalar1=float(DIM))
            nc.vector.reciprocal(out=ninv[:], in_=nrm[:])
            cinv = cpool.tile([G, 1], f32, tag="cinv")
            nc.vector.reciprocal(out=cinv[:], in_=cntc[:])

            x_tiles = [[], []]
            h1_tiles = [[], []]
            pool_psum = None

            for L in range(2):
                htab_l = dram.tile([SPC, ROW], bf16, tag=f"htl{L}")
                htab_g = dram.tile([NSLOT, ROW], bf16, tag=f"htg{L}")
                alD = dram.tile([SPC, H], f32, tag=f"alD{L}")

                # ---------------- Phase A: own rows of h/al tables
                for i in range(TPC):
                    if L == 0:
                        x_t = keep.tile([P, DIM], f32, tag="xk")
                        nc.sync.dma_start(out=x_t[:], in_=x0[i * P:(i + 1) * P, :])
                        x_tiles[0].append(x_t)
                    x_t = x_tiles[L][i]
                    xT_ps = ps.tile([DIM, P], f32, tag="scr")
                    nc.tensor.transpose(xT_ps[:], x_t[:], ident[:])
                    xT = sb.tile([DIM, P], f32, tag="xT")
                    nc.vector.tensor_copy(out=xT[:], in_=xT_ps[:])
                    h_ps = ps.tile([P, HD], f32, tag="hps")
                    nc.tensor.matmul(h_ps[:], lhsT=xT[:], rhs=W_sb[L][:],
                                     start=True, stop=True)
                    row_t = big.tile([P, ROW], bf16, tag="row")
                    nc.vector.tensor_copy(out=row_t[:, 0:HD], in_=h_ps[:])
                    tmp = big.tile([P, HD], bf16, tag="atmp")
                    h3 = row_t[:, 0:HD].rearrange("p (h d) -> p h d", h=H)
                    nc.vector.tensor_mul(out=tmp[:], in0=row_t[:, 0:HD],
                                         in1=asad_sb[L][0][:])
                    als_v = row_t[:, HD:ALS].bitcast(f32)
                    nc.vector.tensor_reduce(
                        out=als_v, in_=tmp[:].rearrange("p (h d) -> p h d", h=H),
                        axis=AX, op=OP.add)
                    nc.vector.tensor_mul(out=tmp[:], in0=row_t[:, 0:HD],
                                         in1=asad_sb[L][1][:])
                    ald_t = sb.tile([P, H], f32, tag="aldA")
                    nc.vector.tensor_reduce(
                        out=ald_t[:], in_=tmp[:].rearrange("p (h d) -> p h d", h=H),
                        axis=AX, op=OP.add)
                    nc.sync.dma_start(out=htab_l[i * P:(i + 1) * P, :], in_=row_t[:])
                    nc.sync.dma_start(out=alD[i * P:(i + 1) * P, :], in_=ald_t[:])

                nc.gpsimd.collective_compute(
                    "AllGather", OP.bypass,
                    replica_groups=[list(range(NC))],
                    ins=[htab_l.opt()], outs=[htab_g.opt()],
                )

                # ---------------- Phase B: aggregate into own dst tiles
                stats_full = statp.tile([G, DIM], f32, tag="sacc")
                stats_psum = stats_full[:, 0:2]
                for i in range(TPC):
                    gix = sb.tile([P, CT], i32, tag="gix")
                    nc.sync.dma_start(out=gix[:], in_=gxd[i])
                    pk = big.tile([P, CT, ROW], bf16, tag="pk")
                    for c in range(CT):
                        nc.gpsimd.indirect_dma_start(
                            out=pk[:, c, :], out_offset=None, in_=htab_g[:],
                            in_offset=bass.IndirectOffsetOnAxis(
                                ap=gix[:, c:c + 1], axis=0))
                    oh_t = big.tile([P, CT * P], bf16, tag="oht")
                    nc.gpsimd.dma_start(out=oh_t[:], in_=ohd[i])
                    ohT_t = big.tile([P, CT * P], bf16, tag="ohTt")
                    nc.gpsimd.dma_start(out=ohT_t[:], in_=ohTd[i])
                    aldt = sb.tile([P, H], f32, tag="aldt")
                    nc.sync.dma_start(out=aldt[:], in_=alD[i * P:(i + 1) * P, :])
                    aldb = sb.tile([P, H], bf16, tag="aldb")
                    nc.vector.tensor_copy(out=aldb[:], in_=aldt[:])

                    acc_h = accp.tile([P, HD], f32, tag="acch")
                    acc_s = accsp.tile([P, H], f32, tag="accs")
                    for c in range(CT):
                        adbc = ps.tile([P, H], f32, tag="scr")
                        nc.tensor.matmul(adbc[:], lhsT=ohT_t[:, c * P:(c + 1) * P],
                                         rhs=aldb[:], start=True, stop=True,
                                         skip_group_check=True)
                        logit = sb.tile([P, H], f32, tag="logit")
                        nc.vector.tensor_add(
                            out=logit[:],
                            in0=pk[:, c, HD:ALS].bitcast(f32),
                            in1=adbc[:])
                        lrl = sb.tile([P, H], f32, tag="lrl")
                        nc.vector.scalar_tensor_tensor(
                            out=lrl[:], in0=logit[:], scalar=cfg["NEG"],
                            in1=logit[:], op0=OP.mult, op1=OP.max)
                        expv = sb.tile([P, H], bf16, tag="expv")
                        nc.scalar.activation(expv[:], lrl[:], ACT.Exp)
                        wh = big.tile([P, H, DIM], bf16, tag="wh")
                        nc.vector.tensor_mul(
                            out=wh[:],
                            in0=pk[:, c, 0:HD].rearrange("p (h d) -> p h d", h=H),
                            in1=expv[:].to_broadcast([P, H, DIM]))
                        lhs = oh_t[:, c * P:(c + 1) * P]
                        nc.tensor.matmul(acc_h[:], lhsT=lhs, rhs=wh[:].rearrange("p h d -> p (h d)"),
                                         start=(c == 0), stop=(c == CT - 1),
                                         skip_group_check=True)
                        nc.tensor.matmul(acc_s[:], lhsT=lhs, rhs=expv[:],
                                         start=(c == 0), stop=(c == CT - 1),
                                         skip_group_check=True)

                    seps = sb.tile([P, H], f32, tag="seps")
                    nc.vector.tensor_scalar_add(out=seps[:], in0=acc_s[:],
                                                scalar1=1e-16)
                    rec = sb.tile([P, H], f32, tag="rec")
                    nc.vector.reciprocal(out=rec[:], in_=seps[:])
                    outf = big.tile([P, H, DIM], f32, tag="outf")
                    nc.vector.tensor_mul(
                        out=outf[:],
                        in0=acc_h[:].rearrange("p (h d) -> p h d", h=H),
                        in1=rec[:].to_broadcast([P, H, DIM]))
                    o2 = outf[:].rearrange("p h d -> p (h d)")
                    t256 = sb.tile([P, 4 * DIM], f32, tag="t256")
                    nc.vector.tensor_add(out=t256[:], in0=o2[:, 0:4 * DIM],
                                         in1=o2[:, 4 * DIM:8 * DIM])
                    t128 = sb.tile([P, 2 * DIM], f32, tag="t128")
                    nc.vector.tensor_add(out=t128[:], in0=t256[:, 0:2 * DIM],
                                         in1=t256[:, 2 * DIM:4 * DIM])
                    t64 = sb.tile([P, DIM], f32, tag="t64")
                    nc.vector.tensor_add(out=t64[:], in0=t128[:, 0:DIM],
                                         in1=t128[:, DIM:2 * DIM])
                    gat = sb.tile([P, DIM], f32, tag="gat")
                    nc.vector.scalar_tensor_tensor(
                        out=gat[:], in0=t64[:], scalar=1.0 / H,
                        in1=bias_sb[L][:], op0=OP.mult, op1=OP.add)
                    h1_t = (keep if L == 0 else keep2).tile([P, DIM], f32,
                                                            tag="h1k")
                    nc.vector.tensor_add(out=h1_t[:], in0=gat[:],
                                         in1=x_tiles[L][i][:])
                    h1_tiles[L].append(h1_t)

                    # LN stats
                    goh_t = sb.tile([P, G], f32, tag="goht")
                    nc.sync.dma_start(out=goh_t[:], in_=goh[i])
                    rr = sb.tile([P, 2], f32, tag="rr")
                    nc.vector.tensor_reduce(out=rr[:, 0:1], in_=h1_t[:],
                                            axis=AX, op=OP.add)
                    sq = sb.tile([P, DIM], f32, tag="sq")
                    nc.vector.tensor_mul(out=sq[:], in0=h1_t[:], in1=h1_t[:])
                    nc.vector.tensor_reduce(out=rr[:, 1:2], in_=sq[:],
                                            axis=AX, op=OP.add)
                    nc.tensor.matmul(stats_psum[:], lhsT=goh_t[:], rhs=rr[:],
                                     start=(i == 0), stop=(i == TPC - 1),
                                     skip_group_check=True)

                # ---------------- AllReduce LN stats; graph params
                st_sb = sb.tile([G, 2], f32, tag="stsb")
                nc.vector.tensor_copy(out=st_sb[:], in_=stats_psum[:])
                st_in = dram.tile([G, 2], f32, tag=f"stin{L}")
                st_out = dram.tile([G, 2], f32, tag=f"stout{L}")
                nc.sync.dma_start(out=st_in[:], in_=st_sb[:])
                nc.gpsimd.collective_compute(
                    "AllReduce", OP.add, replica_groups=[list(range(NC))],
                    ins=[st_in.opt()], outs=[st_out.opt()],
                )
                st_g = sb.tile([G, 2], f32, tag="stg")
                nc.sync.dma_start(out=st_g[:], in_=st_out[:])
                mean = sb.tile([G, 1], f32, tag="mean")
                nc.vector.tensor_mul(out=mean[:], in0=st_g[:, 0:1], in1=ninv[:])
                ex2 = sb.tile([G, 1], f32, tag="ex2")
                nc.vector.tensor_mul(out=ex2[:], in0=st_g[:, 1:2], in1=ninv[:])
                m2 = sb.tile([G, 1], f32, tag="m2")
                nc.vector.tensor_mul(out=m2[:], in0=mean[:], in1=mean[:])
                var = sb.tile([G, 1], f32, tag="var")
                nc.vector.tensor_sub(out=var[:], in0=ex2[:], in1=m2[:])
                nc.vector.tensor_scalar_add(out=var[:], in0=var[:],
                                            scalar1=cfg["EPS"])
                std = sb.tile([G, 1], f32, tag="std")
                nc.scalar.activation(std[:], var[:], ACT.Sqrt)
                stats2 = sb.tile([G, 2], f32, tag="st2")
                nc.vector.tensor_copy(out=stats2[:, 0:1], in_=mean[:])
                nc.vector.reciprocal(out=stats2[:, 1:2], in_=std[:])

                # ---------------- LN apply + ReLU (+ next-layer x / pooling)
                for i in range(TPC):
                    gohT_t = sb.tile([G, P], f32, tag="gohTt")
                    nc.sync.dma_start(out=gohT_t[:], in_=gohT[i])
                    mr_ps = ps.tile([P, 2], f32, tag="scr")
                    nc.tensor.matmul(mr_ps[:], lhsT=gohT_t[:], rhs=stats2[:],
                                     start=True, stop=True,
                                     skip_group_check=True)
                    mr = sb.tile([P, 2], f32, tag="mr")
                    nc.vector.tensor_copy(out=mr[:], in_=mr_ps[:])
                    xn = sb.tile([P, DIM], f32, tag="xn")
                    nc.vector.tensor_scalar(
                        out=xn[:], in0=h1_tiles[L][i][:],
                        scalar1=mr[:, 0:1], scalar2=mr[:, 1:2],
                        op0=OP.subtract, op1=OP.mult)
                    xw = sb.tile([P, DIM], f32, tag="xw")
                    nc.vector.tensor_mul(out=xw[:], in0=xn[:], in1=lnw_sb[L][:])
                    nc.vector.tensor_add(out=xw[:], in0=xw[:], in1=lnb_sb[L][:])
                    if L == 0:
                        x2_t = keep2.tile([P, DIM], f32, tag="xk2")
                        nc.vector.tensor_scalar_max(out=x2_t[:], in0=xw[:],
                                                    scalar1=0.0)
                        x_tiles[1].append(x2_t)
                    else:
                        x3_t = sb.tile([P, DIM], f32, tag="x3")
                        nc.vector.tensor_scalar_max(out=x3_t[:], in0=xw[:],
                                                    scalar1=0.0)
                        goh_t2 = sb.tile([P, G], f32, tag="goht2")
                        nc.sync.dma_start(out=goh_t2[:], in_=goh[i])
                        if pool_psum is None:
                            pool_psum = statp.tile([G, DIM], f32, tag="sacc")
                        nc.tensor.matmul(pool_psum[:], lhsT=goh_t2[:],
                                         rhs=x3_t[:],
                                         start=(i == 0), stop=(i == TPC - 1),
                                         skip_group_check=True)

            # ---------------- head
            pool_sb = sb.tile([G, DIM], f32, tag="poolsb")
            nc.vector.tensor_copy(out=pool_sb[:], in_=pool_psum[:])
            pl_in = dram.tile([G, DIM], f32, tag="plin")
            pl_out = dram.tile([G, DIM], f32, tag="plout")
            nc.sync.dma_start(out=pl_in[:], in_=pool_sb[:])
            nc.gpsimd.collective_compute(
                "AllReduce", OP.add, replica_groups=[list(range(NC))],
                ins=[pl_in.opt()], outs=[pl_out.opt()],
            )
            pool_g = sb.tile([G, DIM], f32, tag="poolg")
            nc.sync.dma_start(out=pool_g[:], in_=pl_out[:])
            pm = sb.tile([G, DIM], f32, tag="pm")
            nc.vector.tensor_scalar_mul(out=pm[:], in0=pool_g[:],
                                        scalar1=cinv[:, 0:1])
            pmT_ps = ps.tile([DIM, G], f32, tag="scr")
            nc.tensor.transpose(pmT_ps[:], pm[:], ident[0:G, 0:G])
            pmT = sb.tile([DIM, G], f32, tag="pmT")
            nc.vector.tensor_copy(out=pmT[:], in_=pmT_ps[:])
            hd_ps = ps.tile([G, OUT], f32, tag="scr")
            nc.tensor.matmul(hd_ps[:], lhsT=pmT[:], rhs=Wh_sb[:],
                             start=True, stop=True, skip_group_check=True)
            res = sb.tile([G, OUT], f32, tag="res")
            nc.vector.tensor_add(out=res[:], in0=hd_ps[:], in1=bh_sb[:])
            nc.vector.tensor_scalar_max(out=res[:], in0=res[:], scalar1=0.0)
            nc.sync.dma_start(out=outd[:, :], in_=res[:])

    nc.compile()
    return nc


# ---------------------------------------------------------------- entry

_CACHE = {}


def _get_nc(cfg, CT):
    key = (tuple(sorted(cfg.items())), CT)
    if key not in _CACHE:
        _CACHE[key] = build(cfg, CT)
    return _CACHE[key]


def _ensure_ntff_hook():
    """Register the axon NTFF profile hook if boot skipped it (best effort).
    The image's antenv package lacks axon_hooks; inject it into sys.modules."""
    try:
        import types
        import antenv
        ah = sys.modules.get("antenv.axon_hooks")
        if ah is None:
            ah = types.ModuleType("antenv.axon_hooks")
            ah._HOOK = None

            def _set(hook, _m=ah):
                _m._HOOK = hook

            def _get(_m=ah):
                return _m._HOOK

            ah.set_axon_ntff_profile_hook = _set
            ah.get_axon_ntff_profile_hook = _get
            sys.modules["antenv.axon_hooks"] = ah
            antenv.axon_hooks = ah
        if ah.get_axon_ntff_profile_hook() is None:
            from trn_agent_boot.trn_boot import _ntff_profile_via_ctypes
            hook = _ntff_profile_via_ctypes("/opt/axon/libaxon_pjrt.so")
            if hook is not None:
                ah.set_axon_ntff_profile_hook(hook)
    except Exception as e:
        print(f"[ntff hook] registration failed: {type(e).__name__}: {e}")


def run(inputs, cfg=None, trace=False):
    from concourse.bass_utils import run_bass_kernel_spmd
    if trace:
        _ensure_ntff_hook()
    cfg = cfg or full_cfg()
    weights = {k: inputs[k] for k in
               ["W1", "a_src1", "a_dst1", "b1", "ln1_w", "ln1_b",
                "W2", "a_src2", "a_dst2", "b2", "ln2_w", "ln2_b", "Wh", "bh"]}
    in_maps, CT = prep(cfg, inputs["x"], inputs["edge_index"],
                       inputs["batch"], weights)
    nc = _get_nc(cfg, CT)
    res = run_bass_kernel_spmd(nc, in_maps, list(range(cfg["NC"])),
                               trace=trace)
    out = np.asarray(res.results[0]["out"], np.float32)
    return out, res


def kernel(**inputs):
    out, _ = run(inputs, trace=False)
    return out


# revision 11
# speedup vs baseline: 1.0537x; 1.0537x over previous
"""DynamicGAT (2-layer GAT + graph-LN + mean-pool head) on 8 TRN2 NeuronCores.

Strategy (dst-sharded message passing):
  - Host (numpy, integer/layout work only): add self loops, balance nodes
    into 128-slot dst tiles (greedy LPT on in-degree), permute node ids,
    bucket edges per dst tile padded to Cmax chunks of 128 edges, build
    per-chunk one-hot (edge -> dst-slot) int8 matrices, graph one-hots,
    and broadcast-materialized weight tiles.
  - Device, per layer:
      Phase A (own node slice): h = x @ W (PE, via PE-transpose of x),
        attention logit halves al_s/al_d (DVE mul+reduce), pack
        [h:bf16[512] | al_s:f32[8]] rows, write local table + al_d table.
      AllGather the packed table (8 cores, ~7MB/rank).
      Phase B (own dst tiles): batched indirect-DMA gather of packed rows
        by edge src, gather al_d rows by edge dst, per-chunk softmax
        (no max-subtraction; logits are O(1)) and exp-weighted rows,
        PSUM-accumulated one-hot matmul aggregation (values + denominator),
        then normalize, head-mean, bias, residual.
      Graph LayerNorm: per-graph sums via one-hot matmul into a persistent
        PSUM accumulator, tiny AllReduce, per-node broadcast via transposed
        one-hot matmul, normalize + affine + ReLU (DVE).
  - Head: per-graph mean-pool via one-hot matmul + AllReduce, tiny matmul,
    bias + ReLU.
"""

import math
import os
import sys

import numpy as np

sys.path.insert(0, "/opt/trn_rl_repo")

P = 128


def full_cfg():
    return dict(
        N=50000, E=400000, G=64, DIM=64, HEADS=8, OUT=32,
        NC=8, TPC=49, NEG=0.2, EPS=1e-5,
    )


# ---------------------------------------------------------------- host prep

def _balance_nodes(deg, T):
    """Assign nodes to T tiles of <=P slots, balancing total in-degree.
    Returns slot_of_node [N] (global slot id = tile*P + slot)."""
    import heapq
    N = deg.shape[0]
    order = np.argsort(-deg, kind="stable")
    heap = [(0, t) for t in range(T)]
    heapq.heapify(heap)
    counts = np.zeros(T, np.int64)
    slot_of_node = np.empty(N, np.int64)
    for n in order:
        while True:
            load, t = heapq.heappop(heap)
            if counts[t] < P:
                break
        slot_of_node[n] = t * P + counts[t]
        counts[t] += 1
        if counts[t] < P:
            heapq.heappush(heap, (load + int(deg[n]), t))
    return slot_of_node


def prep(cfg, x, edge_index, batch, weights):
    """weights: dict with W1,a_src1,a_dst1,b1,ln1_w,ln1_b, W2,..., Wh,bh.
    Returns (in_maps, Cmax)."""
    N, E, G = cfg["N"], cfg["E"], cfg["G"]
    NC, TPC, DIM, H = cfg["NC"], cfg["TPC"], cfg["DIM"], cfg["HEADS"]
    HD = H * DIM
    T = NC * TPC
    NSLOT = T * P
    SPC = TPC * P  # slots per core

    x = np.asarray(x, np.float32)
    ei = np.asarray(edge_index, np.int64)
    batch = np.asarray(batch, np.int64)

    loops = np.arange(N, dtype=np.int64)
    src = np.concatenate([ei[0], loops])
    dst = np.concatenate([ei[1], loops])
    deg = np.bincount(dst, minlength=N)

    slot_of_node = _balance_nodes(deg, T)

    # per-tile edge buckets (Cmax chunks of 128 edges, zero-padded with src slot 0)
    e_tile = slot_of_node[dst] // P
    ord_e = np.argsort(e_tile, kind="stable")
    src_s, dst_s = src[ord_e], dst[ord_e]
    e_tile_s = e_tile[ord_e]
    tile_cnt = np.bincount(e_tile_s, minlength=T)
    CT = int(math.ceil(tile_cnt.max() / P))

    import ml_dtypes
    bf = ml_dtypes.bfloat16
    gidx = np.zeros((T, P, CT), np.int32)     # src slot per (edge p, chunk c)
    oh = np.zeros((T, P, CT * P), bf)         # [edge p, c*128 + dloc]
    ohT = np.zeros((T, P, CT * P), bf)        # [dloc, c*128 + edge p]
    starts = np.concatenate([[0], np.cumsum(tile_cnt)])
    for t in range(T):
        s, e = starts[t], starts[t + 1]
        sslot = slot_of_node[src_s[s:e]]
        dloc = slot_of_node[dst_s[s:e]] % P
        k = np.arange(e - s)
        c, p = k // P, k % P
        gidx[t, p, c] = sslot
        oh[t, p, c * P + dloc] = 1
        ohT[t, dloc, c * P + p] = 1

    # graph one-hots (zero rows for empty slots)
    node_of_slot = np.full(NSLOT, -1, np.int64)
    node_of_slot[slot_of_node] = np.arange(N)
    goh = np.zeros((T, P, G), np.float32)
    tt, ss = np.divmod(np.arange(NSLOT), P)
    valid = node_of_slot >= 0
    goh[tt[valid], ss[valid], batch[node_of_slot[valid]]] = 1.0
    gohT = np.ascontiguousarray(np.transpose(goh, (0, 2, 1)))

    xp = np.zeros((NSLOT, DIM), np.float32)
    xp[slot_of_node] = x

    cnt = np.bincount(batch, minlength=G).astype(np.float32).reshape(G, 1)

    def bc_row(a, reps=P):  # [k] -> [reps, k]
        return np.ascontiguousarray(
            np.broadcast_to(np.asarray(a, np.float32).reshape(1, -1), (reps, a.size)))

    w = {k: np.asarray(v, np.float32) for k, v in weights.items()}
    common = dict(
        W1t=w["W1"], W2t=w["W2"],
        as1=bc_row(w["a_src1"].reshape(HD)), ad1=bc_row(w["a_dst1"].reshape(HD)),
        as2=bc_row(w["a_src2"].reshape(HD)), ad2=bc_row(w["a_dst2"].reshape(HD)),
        b1t=bc_row(w["b1"]), b2t=bc_row(w["b2"]),
        lnw1=bc_row(w["ln1_w"]), lnb1=bc_row(w["ln1_b"]),
        lnw2=bc_row(w["ln2_w"]), lnb2=bc_row(w["ln2_b"]),
        Wht=w["Wh"], bht=bc_row(w["bh"], reps=G), cntt=cnt,
    )

    in_maps = []
    for c in range(NC):
        tl = slice(c * TPC, (c + 1) * TPC)
        m = dict(common)
        m["x0"] = np.ascontiguousarray(xp[c * SPC:(c + 1) * SPC])
        m["gidx"] = np.ascontiguousarray(gidx[tl])
        m["oh"] = np.ascontiguousarray(oh[tl])
        m["ohT"] = np.ascontiguousarray(ohT[tl])
        m["goh"] = np.ascontiguousarray(goh[tl])
        m["gohT"] = np.ascontiguousarray(gohT[tl])
        in_maps.append(m)
    return in_maps, CT


# ---------------------------------------------------------------- device

def build(cfg, CT):
    import concourse.bacc as bacc
    import concourse.bass as bass
    import concourse.tile as tile
    from concourse import mybir
    from concourse.masks import make_identity

    N, G = cfg["N"], cfg["G"]
    NC, TPC, DIM, H, OUT = cfg["NC"], cfg["TPC"], cfg["DIM"], cfg["HEADS"], cfg["OUT"]
    HD = H * DIM            # 512
    ALS = HD + 2 * H        # 528
    ROW = ALS               # bf16 slots per packed table row (1056B)
    T = NC * TPC
    NSLOT = T * P
    SPC = TPC * P
    f32 = mybir.dt.float32
    bf16 = mybir.dt.bfloat16
    i32 = mybir.dt.int32
    i16 = mybir.dt.int16
    i8 = mybir.dt.int8
    AX = mybir.AxisListType.X
    OP = mybir.AluOpType
    ACT = mybir.ActivationFunctionType

    nc = bacc.Bacc("TRN2", target_bir_lowering=False, debug=False,
                   enable_asserts=False, num_devices=NC)

    def din(name, shape, dt):
        return nc.dram_tensor(name, shape, dt, kind="ExternalInput")

    x0 = din("x0", [SPC, DIM], f32)
    W1t, W2t = din("W1t", [DIM, HD], f32), din("W2t", [DIM, HD], f32)
    as1, ad1 = din("as1", [P, HD], f32), din("ad1", [P, HD], f32)
    as2, ad2 = din("as2", [P, HD], f32), din("ad2", [P, HD], f32)
    b1t, b2t = din("b1t", [P, DIM], f32), din("b2t", [P, DIM], f32)
    lnw1, lnb1 = din("lnw1", [P, DIM], f32), din("lnb1", [P, DIM], f32)
    lnw2, lnb2 = din("lnw2", [P, DIM], f32), din("lnb2", [P, DIM], f32)
    Wht, bht = din("Wht", [DIM, OUT], f32), din("bht", [G, OUT], f32)
    cntt = din("cntt", [G, 1], f32)
    gxd = din("gidx", [TPC, P, CT], i32)
    ohd = din("oh", [TPC, P, CT * P], bf16)
    ohTd = din("ohT", [TPC, P, CT * P], bf16)
    goh = din("goh", [TPC, P, G], f32)
    gohT = din("gohT", [TPC, G, P], f32)
    outd = nc.dram_tensor("out", [G, OUT], f32, kind="ExternalOutput")

    with tile.TileContext(nc) as tc:
        with (
            tc.tile_pool(name="const", bufs=1) as cpool,
            tc.tile_pool(name="sb", bufs=3) as sb,
            tc.tile_pool(name="big", bufs=2) as big,
            tc.tile_pool(name="keep", bufs=2 * TPC) as keep,
            tc.tile_pool(name="keep2", bufs=2 * TPC) as keep2,
            tc.tile_pool(name="ps", bufs=1, space="PSUM") as ps,
            tc.tile_pool(name="acc", bufs=2, space="PSUM") as accp,
            tc.tile_pool(name="accs", bufs=2, space="PSUM") as accsp,
            tc.tile_pool(name="stat", bufs=1, space="PSUM") as statp,
            tc.tile_pool(name="dram", bufs=1, space="DRAM") as dram,
        ):
            ident = cpool.tile([P, P], f32, tag="ident")
            make_identity(nc, ident[:])

            # constants to SBUF (bf16 casts for attention vectors)
            def load_const(dt_dram, shape, dt=f32, tag=None):
                t = cpool.tile(shape, dt, tag=tag)
                nc.sync.dma_start(out=t[:], in_=dt_dram[:])
                return t

            W_sb = [load_const(W1t, [DIM, HD], tag="W1"),
                    load_const(W2t, [DIM, HD], tag="W2")]
            asad_sb = []
            for L, (a_s, a_d) in enumerate([(as1, ad1), (as2, ad2)]):
                pair = []
                for nm, dd in (("s", a_s), ("d", a_d)):
                    t32 = sb.tile([P, HD], f32, tag="acast")
                    nc.sync.dma_start(out=t32[:], in_=dd[:])
                    tb = cpool.tile([P, HD], bf16, tag=f"a{nm}{L}")
                    nc.vector.tensor_copy(out=tb[:], in_=t32[:])
                    pair.append(tb)
                asad_sb.append(pair)
            bias_sb = [load_const(b1t, [P, DIM], tag="b1"),
                       load_const(b2t, [P, DIM], tag="b2")]
            lnw_sb = [load_const(lnw1, [P, DIM], tag="lnw1"),
                      load_const(lnw2, [P, DIM], tag="lnw2")]
            lnb_sb = [load_const(lnb1, [P, DIM], tag="lnb1"),
                      load_const(lnb2, [P, DIM], tag="lnb2")]
            Wh_sb = load_const(Wht, [DIM, OUT], tag="Wh")
            bh_sb = load_const(bht, [G, OUT], tag="bh")
            cnt_sb = load_const(cntt, [G, 1], tag="cnt")
            # clipped count and 1/(cnt*DIM)
            cntc = cpool.tile([G, 1], f32, tag="cntc")
            nc.vector.tensor_scalar_max(out=cntc[:], in0=cnt_sb[:], scalar1=1.0)
            ninv = cpool.tile([G, 1], f32, tag="ninv")
            nrm = sb.tile([G, 1], f32, tag="nrm")
            nc.vector.tensor_scalar_mul(out=nrm[:], in0=cntc[:], scalar1=float(DIM))
            nc.vector.reciprocal(out=ninv[:], in_=nrm[:])
            cinv = cpool.tile([G, 1], f32, tag="cinv")
            nc.vector.reciprocal(out=cinv[:], in_=cntc[:])

            x_tiles = [[], []]
            h1_tiles = [[], []]
            pool_psum = None

            for L in range(2):
                htab_l = dram.tile([SPC, ROW], bf16, tag=f"htl{L}")
                htab_g = dram.tile([NSLOT, ROW], bf16, tag=f"htg{L}")
                alD = dram.tile([SPC, H], f32, tag=f"alD{L}")

                # ---------------- Phase A: own rows of h/al tables
                for i in range(TPC):
                    if L == 0:
                        x_t = keep.tile([P, DIM], f32, tag="xk")
                        nc.sync.dma_start(out=x_t[:], in_=x0[i * P:(i + 1) * P, :])
                        x_tiles[0].append(x_t)
                    x_t = x_tiles[L][i]
                    xT_ps = ps.tile([DIM, P], f32, tag="scr")
                    nc.tensor.transpose(xT_ps[:], x_t[:], ident[:])
                    xT = sb.tile([DIM, P], f32, tag="xT")
                    nc.vector.tensor_copy(out=xT[:], in_=xT_ps[:])
                    h_ps = ps.tile([P, HD], f32, tag="hps")
                    nc.tensor.matmul(h_ps[:], lhsT=xT[:], rhs=W_sb[L][:],
                                     start=True, stop=True)
                    row_t = big.tile([P, ROW], bf16, tag="row")
                    nc.vector.tensor_copy(out=row_t[:, 0:HD], in_=h_ps[:])
                    tmp = big.tile([P, HD], bf16, tag="atmp")
                    h3 = row_t[:, 0:HD].rearrange("p (h d) -> p h d", h=H)
                    nc.vector.tensor_mul(out=tmp[:], in0=row_t[:, 0:HD],
                                         in1=asad_sb[L][0][:])
                    als_v = row_t[:, HD:ALS].bitcast(f32)
                    nc.vector.tensor_reduce(
                        out=als_v, in_=tmp[:].rearrange("p (h d) -> p h d", h=H),
                        axis=AX, op=OP.add)
                    nc.vector.tensor_mul(out=tmp[:], in0=row_t[:, 0:HD],
                                         in1=asad_sb[L][1][:])
                    ald_t = sb.tile([P, H], f32, tag="aldA")
                    nc.vector.tensor_reduce(
                        out=ald_t[:], in_=tmp[:].rearrange("p (h d) -> p h d", h=H),
                        axis=AX, op=OP.add)
                    nc.sync.dma_start(out=htab_l[i * P:(i + 1) * P, :], in_=row_t[:])
                    nc.sync.dma_start(out=alD[i * P:(i + 1) * P, :], in_=ald_t[:])

                nc.gpsimd.collective_compute(
                    "AllGather", OP.bypass,
                    replica_groups=[list(range(NC))],
                    ins=[htab_l.opt()], outs=[htab_g.opt()],
                )

                # ---------------- Phase B: aggregate into own dst tiles
                stats_full = statp.tile([G, DIM], f32, tag="sacc")
                stats_psum = stats_full[:, 0:2]
                for i in range(TPC):
                    gix = sb.tile([P, CT], i32, tag="gix")
                    nc.sync.dma_start(out=gix[:], in_=gxd[i])
                    pk = big.tile([P, CT, ROW], bf16, tag="pk")
                    for c in range(CT):
                        nc.gpsimd.indirect_dma_start(
                            out=pk[:, c, :], out_offset=None, in_=htab_g[:],
                            in_offset=bass.IndirectOffsetOnAxis(
                                ap=gix[:, c:c + 1], axis=0))
                    oh_t = big.tile([P, CT * P], bf16, tag="oht")
                    nc.sync.dma_start(out=oh_t[:], in_=ohd[i])
                    ohT_t = big.tile([P, CT * P], bf16, tag="ohTt")
                    nc.sync.dma_start(out=ohT_t[:], in_=ohTd[i])
                    aldt = sb.tile([P, H], f32, tag="aldt")
                    nc.sync.dma_start(out=aldt[:], in_=alD[i * P:(i + 1) * P, :])
                    aldb = sb.tile([P, H], bf16, tag="aldb")
                    nc.vector.tensor_copy(out=aldb[:], in_=aldt[:])

                    acc_h = accp.tile([P, HD], f32, tag="acch")
                    acc_s = accsp.tile([P, H], f32, tag="accs")
                    for c in range(CT):
                        adbc = ps.tile([P, H], f32, tag="scr")
                        nc.tensor.matmul(adbc[:], lhsT=ohT_t[:, c * P:(c + 1) * P],
                                         rhs=aldb[:], start=True, stop=True,
                                         skip_group_check=True)
                        logit = sb.tile([P, H], f32, tag="logit")
                        nc.vector.tensor_add(
                            out=logit[:],
                            in0=pk[:, c, HD:ALS].bitcast(f32),
                            in1=adbc[:])
                        lrl = sb.tile([P, H], f32, tag="lrl")
                        nc.vector.scalar_tensor_tensor(
                            out=lrl[:], in0=logit[:], scalar=cfg["NEG"],
                            in1=logit[:], op0=OP.mult, op1=OP.max)
                        expv = sb.tile([P, H], bf16, tag="expv")
                        nc.scalar.activation(expv[:], lrl[:], ACT.Exp)
                        wh = big.tile([P, H, DIM], bf16, tag="wh")
                        nc.vector.tensor_mul(
                            out=wh[:],
                            in0=pk[:, c, 0:HD].rearrange("p (h d) -> p h d", h=H),
                            in1=expv[:].to_broadcast([P, H, DIM]))
                        lhs = oh_t[:, c * P:(c + 1) * P]
                        nc.tensor.matmul(acc_h[:], lhsT=lhs, rhs=wh[:].rearrange("p h d -> p (h d)"),
                                         start=(c == 0), stop=(c == CT - 1),
                                         skip_group_check=True)
                        nc.tensor.matmul(acc_s[:], lhsT=lhs, rhs=expv[:],
                                         start=(c == 0), stop=(c == CT - 1),
                                         skip_group_check=True)

                    seps = sb.tile([P, H], f32, tag="seps")
                    nc.vector.tensor_scalar_add(out=seps[:], in0=acc_s[:],
                                                scalar1=1e-16)
                    rec = sb.tile([P, H], f32, tag="rec")
                    nc.vector.reciprocal(out=rec[:], in_=seps[:])
                    outf = big.tile([P, H, DIM], f32, tag="outf")
                    nc.vector.tensor_mul(
                        out=outf[:],
                        in0=acc_h[:].rearrange("p (h d) -> p h d", h=H),
                        in1=rec[:].to_broadcast([P, H, DIM]))
                    o2 = outf[:].rearrange("p h d -> p (h d)")
                    t256 = sb.tile([P, 4 * DIM], f32, tag="t256")
                    nc.vector.tensor_add(out=t256[:], in0=o2[:, 0:4 * DIM],
                                         in1=o2[:, 4 * DIM:8 * DIM])
                    t128 = sb.tile([P, 2 * DIM], f32, tag="t128")
                    nc.vector.tensor_add(out=t128[:], in0=t256[:, 0:2 * DIM],
                                         in1=t256[:, 2 * DIM:4 * DIM])
                    t64 = sb.tile([P, DIM], f32, tag="t64")
                    nc.vector.tensor_add(out=t64[:], in0=t128[:, 0:DIM],
                                         in1=t128[:, DIM:2 * DIM])
                    gat = sb.tile([P, DIM], f32, tag="gat")
                    nc.vector.scalar_tensor_tensor(
                        out=gat[:], in0=t64[:], scalar=1.0 / H,
                        in1=bias_sb[L][:], op0=OP.mult, op1=OP.add)
                    h1_t = (keep if L == 0 else keep2).tile([P, DIM], f32,
                                                            tag="h1k")
                    nc.vector.tensor_add(out=h1_t[:], in0=gat[:],
                                         in1=x_tiles[L][i][:])
                    h1_tiles[L].append(h1_t)

                    # LN stats
                    goh_t = sb.tile([P, G], f32, tag="goht")
                    nc.sync.dma_start(out=goh_t[:], in_=goh[i])
                    rr = sb.tile([P, 2], f32, tag="rr")
                    nc.vector.tensor_reduce(out=rr[:, 0:1], in_=h1_t[:],
                                            axis=AX, op=OP.add)
                    sq = sb.tile([P, DIM], f32, tag="sq")
                    nc.vector.tensor_mul(out=sq[:], in0=h1_t[:], in1=h1_t[:])
                    nc.vector.tensor_reduce(out=rr[:, 1:2], in_=sq[:],
                                            axis=AX, op=OP.add)
                    nc.tensor.matmul(stats_psum[:], lhsT=goh_t[:], rhs=rr[:],
                                     start=(i == 0), stop=(i == TPC - 1),
                                     skip_group_check=True)

                # ---------------- AllReduce LN stats; graph params
                st_sb = sb.tile([G, 2], f32, tag="stsb")
                nc.vector.tensor_copy(out=st_sb[:], in_=stats_psum[:])
                st_in = dram.tile([G, 2], f32, tag=f"stin{L}")
                st_out = dram.tile([G, 2], f32, tag=f"stout{L}")
                nc.sync.dma_start(out=st_in[:], in_=st_sb[:])
                nc.gpsimd.collective_compute(
                    "AllReduce", OP.add, replica_groups=[list(range(NC))],
                    ins=[st_in.opt()], outs=[st_out.opt()],
                )
                st_g = sb.tile([G, 2], f32, tag="stg")
                nc.sync.dma_start(out=st_g[:], in_=st_out[:])
                mean = sb.tile([G, 1], f32, tag="mean")
                nc.vector.tensor_mul(out=mean[:], in0=st_g[:, 0:1], in1=ninv[:])
                ex2 = sb.tile([G, 1], f32, tag="ex2")
                nc.vector.tensor_mul(out=ex2[:], in0=st_g[:, 1:2], in1=ninv[:])
                m2 = sb.tile([G, 1], f32, tag="m2")
                nc.vector.tensor_mul(out=m2[:], in0=mean[:], in1=mean[:])
                var = sb.tile([G, 1], f32, tag="var")
                nc.vector.tensor_sub(out=var[:], in0=ex2[:], in1=m2[:])
                nc.vector.tensor_scalar_add(out=var[:], in0=var[:],
                                            scalar1=cfg["EPS"])
                std = sb.tile([G, 1], f32, tag="std")
                nc.scalar.activation(std[:], var[:], ACT.Sqrt)
                stats2 = sb.tile([G, 2], f32, tag="st2")
                nc.vector.tensor_copy(out=stats2[:, 0:1], in_=mean[:])
                nc.vector.reciprocal(out=stats2[:, 1:2], in_=std[:])

                # ---------------- LN apply + ReLU (+ next-layer x / pooling)
                for i in range(TPC):
                    gohT_t = sb.tile([G, P], f32, tag="gohTt")
                    nc.sync.dma_start(out=gohT_t[:], in_=gohT[i])
                    mr_ps = ps.tile([P, 2], f32, tag="scr")
                    nc.tensor.matmul(mr_ps[:], lhsT=gohT_t[:], rhs=stats2[:],
                                     start=True, stop=True,
                                     skip_group_check=True)
                    mr = sb.tile([P, 2], f32, tag="mr")
                    nc.vector.tensor_copy(out=mr[:], in_=mr_ps[:])
                    xn = sb.tile([P, DIM], f32, tag="xn")
                    nc.vector.tensor_scalar(
                        out=xn[:], in0=h1_tiles[L][i][:],
                        scalar1=mr[:, 0:1], scalar2=mr[:, 1:2],
                        op0=OP.subtract, op1=OP.mult)
                    xw = sb.tile([P, DIM], f32, tag="xw")
                    nc.vector.tensor_mul(out=xw[:], in0=xn[:], in1=lnw_sb[L][:])
                    nc.vector.tensor_add(out=xw[:], in0=xw[:], in1=lnb_sb[L][:])
                    if L == 0:
                        x2_t = keep2.tile([P, DIM], f32, tag="xk2")
                        nc.vector.tensor_scalar_max(out=x2_t[:], in0=xw[:],
                                                    scalar1=0.0)
                        x_tiles[1].append(x2_t)
                    else:
                        x3_t = sb.tile([P, DIM], f32, tag="x3")
                        nc.vector.tensor_scalar_max(out=x3_t[:], in0=xw[:],
                                                    scalar1=0.0)
                        goh_t2 = sb.tile([P, G], f32, tag="goht2")
                        nc.sync.dma_start(out=goh_t2[:], in_=goh[i])
                        if pool_psum is None:
                            pool_psum = statp.tile([G, DIM], f32, tag="sacc")
                        nc.tensor.matmul(pool_psum[:], lhsT=goh_t2[:],
                                         rhs=x3_t[:],
                                         start=(i == 0), stop=(i == TPC - 1),
                                         skip_group_check=True)

            # ---------------- head
            pool_sb = sb.tile([G, DIM], f32, tag="poolsb")
            nc.vector.tensor_copy(out=pool_sb[:], in_=pool_psum[:])
            pl_in = dram.tile([G, DIM], f32, tag="plin")
            pl_out = dram.tile([G, DIM], f32, tag="plout")
            nc.sync.dma_start(out=pl_in[:], in_=pool_sb[:])
            nc.gpsimd.collective_compute(
                "AllReduce", OP.add, replica_groups=[list(range(NC))],
                ins=[pl_in.opt()], outs=[pl_out.opt()],
            )
            pool_g = sb.tile([G, DIM], f32, tag="poolg")
            nc.sync.dma_start(out=pool_g[:], in_=pl_out[:])
            pm = sb.tile([G, DIM], f32, tag="pm")
            nc.vector.tensor_scalar_mul(out=pm[:], in0=pool_g[:],
                                        scalar1=cinv[:, 0:1])
            pmT_ps = ps.tile([DIM, G], f32, tag="scr")
            nc.tensor.transpose(pmT_ps[:], pm[:], ident[0:G, 0:G])
            pmT = sb.tile([DIM, G], f32, tag="pmT")
            nc.vector.tensor_copy(out=pmT[:], in_=pmT_ps[:])
            hd_ps = ps.tile([G, OUT], f32, tag="scr")
            nc.tensor.matmul(hd_ps[:], lhsT=pmT[:], rhs=Wh_sb[:],
                             start=True, stop=True, skip_group_check=True)
            res = sb.tile([G, OUT], f32, tag="res")
            nc.vector.tensor_add(out=res[:], in0=hd_ps[:], in1=bh_sb[:])
            nc.vector.tensor_scalar_max(out=res[:], in0=res[:], scalar1=0.0)
            nc.sync.dma_start(out=outd[:, :], in_=res[:])

    nc.compile()
    return nc


# ---------------------------------------------------------------- entry

_CACHE = {}


def _get_nc(cfg, CT):
    key = (tuple(sorted(cfg.items())), CT)
    if key not in _CACHE:
        _CACHE[key] = build(cfg, CT)
    return _CACHE[key]


def _ensure_ntff_hook():
    """Register the axon NTFF profile hook if boot skipped it (best effort).
    The image's antenv package lacks axon_hooks; inject it into sys.modules."""
    try:
        import types
        import antenv
        ah = sys.modules.get("antenv.axon_hooks")
        if ah is None:
            ah = types.ModuleType("antenv.axon_hooks")
            ah._HOOK = None

            def _set(hook, _m=ah):
                _m._HOOK = hook

            def _get(_m=ah):
                return _m._HOOK

            ah.set_axon_ntff_profile_hook = _set
            ah.get_axon_ntff_profile_hook = _get
            sys.modules["antenv.axon_hooks"] = ah
            antenv.axon_hooks = ah
        if ah.get_axon_ntff_profile_hook() is None:
            from trn_agent_boot.trn_boot import _ntff_profile_via_ctypes
            hook = _ntff_profile_via_ctypes("/opt/axon/libaxon_pjrt.so")
            if hook is not None:
                ah.set_axon_ntff_profile_hook(hook)
    except Exception as e:
        print(f"[ntff hook] registration failed: {type(e).__name__}: {e}")


def run(inputs, cfg=None, trace=False):
    from concourse.bass_utils import run_bass_kernel_spmd
    if trace:
        _ensure_ntff_hook()
    cfg = cfg or full_cfg()
    weights = {k: inputs[k] for k in
               ["W1", "a_src1", "a_dst1", "b1", "ln1_w", "ln1_b",
                "W2", "a_src2", "a_dst2", "b2", "ln2_w", "ln2_b", "Wh", "bh"]}
    in_maps, CT = prep(cfg, inputs["x"], inputs["edge_index"],
                       inputs["batch"], weights)
    nc = _get_nc(cfg, CT)
    res = run_bass_kernel_spmd(nc, in_maps, list(range(cfg["NC"])),
                               trace=trace)
    out = np.asarray(res.results[0]["out"], np.float32)
    return out, res


def kernel(**inputs):
    out, _ = run(inputs, trace=False)
    return out
